# revision 4
# baseline (speedup 1.0000x reference)
# CopyGenerator kernel for 8 TRN2 NeuronCores (Bass/Tile, SPMD).
#
# reference computation:
#   logits = hidden @ W.T + b                      [B=1024, V=50000]
#   mod_logits = logits with col COPY(4) = 1e-10
#   prob = softmax(mod_logits); copy = sigmoid(logits[:, 4])
#   out_prob = prob*(1-copy); out_prob[b, alignment[src[b,s]]] += attn[b,s]*copy[b]
#   out_prob[:, 0] = EPS; norm = out_prob.sum(-1)
#   out = log(out_prob/norm + EPS)
#
# Strategy: tensor-parallel over the vocab dim (each core owns VC=6250 columns
# of W / the output; reads its W shard exactly once).  Batch rows live on SBUF
# partitions (8 batch tiles of 128 rows).  Per-row softmax statistics
# (sum_exp, logits[:,4], exp(mod_logits)[:,0]) are combined across cores with
# one tiny 12 KB AllReduce.  The per-row scatter-add is reformulated in the
# exp domain:
#   out[b,v] = ln(alpha[b]*exp(mod_logits[b,v]) + beta[b]*val[b,v] + EPS)
#            = ln(alpha[b]*(exp(mod_logits) + gamma[b]*val) + EPS)
#   alpha = (1-copy)/(sum_exp*norm), gamma = copy*sum_exp/(1-copy)
# where val[b,v] = sum_s attn[b,s]*[alignment[src[b,s]] == v] is input-only and
# precomputed (dense, bf16) on the host as part of sharding.
import numpy as np
import ml_dtypes

import concourse.bacc as bacc
import concourse.bass as bass
import concourse.mybir as mybir
import concourse.tile as tile
from concourse import bass_utils

FP32 = mybir.dt.float32
BF16 = mybir.dt.bfloat16
AF = mybir.ActivationFunctionType
ALU = mybir.AluOpType

B, S, H, V = 1024, 128, 1024, 50000
NCORES = 8
VC = V // NCORES          # 6250 vocab columns per core
NBT = B // 128            # 8 batch tiles of 128 rows
KC = H // 128             # 8 contraction chunks of 128
COPY, PAD, EPS = 4, 0, 1e-10

CHUNK = 512
CHUNKS = [(i * CHUNK, CHUNK) for i in range(VC // CHUNK)]
if VC % CHUNK:
    CHUNKS.append(((VC // CHUNK) * CHUNK, VC % CHUNK))
NCH = len(CHUNKS)

# pass-2 half-tile split; first boundary must be 4-byte aligned in bf16
HALVES = [(0, 3200), (3200, VC - 3200)]


def build_nc(debug: bool = False):
    nc = bacc.Bacc(
        "TRN2", target_bir_lowering=False, debug=debug, num_devices=NCORES
    )
    wt_d = nc.dram_tensor("wt", [H, VC], BF16, kind="ExternalInput")
    ht_d = nc.dram_tensor("ht", [H, B], BF16, kind="ExternalInput")
    b_d = nc.dram_tensor("bias", [1, VC], BF16, kind="ExternalInput")
    val_d = nc.dram_tensor("val", [B, VC], BF16, kind="ExternalInput")
    anz_d = nc.dram_tensor("anz", [128, NBT], FP32, kind="ExternalInput")
    m4_d = nc.dram_tensor("m4", [128, 1], FP32, kind="ExternalInput")
    im4_d = nc.dram_tensor("im4", [128, 1], FP32, kind="ExternalInput")
    ones_d = nc.dram_tensor("ones", [1, 128], BF16, kind="ExternalInput")
    out_d = nc.dram_tensor("out", [B, VC], FP32, kind="ExternalOutput")

    wt_ap = wt_d.ap().rearrange("(k p) v -> p k v", p=128)
    ht_ap = ht_d.ap().rearrange("(k p) b -> p k b", p=128)

    with tile.TileContext(nc) as tc:
        with (
            tc.tile_pool(name="const", bufs=1) as const,
            tc.tile_pool(name="wtp", bufs=2) as wtp,
            tc.tile_pool(name="valp", bufs=3) as valp,
            tc.tile_pool(name="stg", bufs=3) as stg,
            tc.tile_pool(name="ps", bufs=4, space="PSUM") as psp,
            tc.tile_pool(name="dram", bufs=1, space="DRAM") as dram,
        ):
            # ---- resident tensors -------------------------------------
            ht_sb = const.tile([128, KC, B], BF16, tag="ht")
            nc.sync.dma_start(ht_sb[:, :, :], ht_ap)
            b_sb = const.tile([1, VC], BF16, tag="bias")
            nc.sync.dma_start(b_sb[:, :], b_d.ap())
            ones_sb = const.tile([1, 128], BF16, tag="ones")
            nc.sync.dma_start(ones_sb[:, :], ones_d.ap())
            m4_sb = const.tile([128, 1], FP32, tag="m4")
            nc.sync.dma_start(m4_sb[:, :], m4_d.ap())
            im4_sb = const.tile([128, 1], FP32, tag="im4")
            nc.sync.dma_start(im4_sb[:, :], im4_d.ap())
            anz_sb = const.tile([128, NBT], FP32, tag="anz")
            nc.sync.dma_start(anz_sb[:, :], anz_d.ap())

            eps_sb = const.tile([128, 1], FP32, tag="eps")
            nc.vector.memset(eps_sb[:, :], EPS)

            exp_sb = const.tile([128, NBT, VC], BF16, tag="exp")
            l4_sb = const.tile([128, NBT], FP32, tag="l4")
            part_sb = const.tile([128, NBT, NCH], FP32, tag="part")
            ccin_sb = const.tile([128, 3, NBT], FP32, tag="ccin")
            sall_sb = const.tile([128, 3, NBT], FP32, tag="sall")

            # ---- pass 1: logits -> exp(mod_logits), partial row sums ---
            for ci, (c0, cw) in enumerate(CHUNKS):
                wt_t = wtp.tile([128, KC, cw], BF16, tag="wt")
                nc.sync.dma_start(wt_t[:, :, :], wt_ap[:, :, c0 : c0 + cw])
                for j in range(NBT):
                    ps = psp.tile([128, cw], FP32, tag="ps")
                    for k in range(KC):
                        nc.tensor.matmul(
                            ps[:, :],
                            lhsT=ht_sb[:, k, j * 128 : (j + 1) * 128],
                            rhs=wt_t[:, k, :],
                            start=(k == 0),
                            stop=False,
                        )
                    nc.tensor.matmul(
                        ps[:, :],
                        lhsT=ones_sb[:, :],
                        rhs=b_sb[:, c0 : c0 + cw],
                        start=False,
                        stop=True,
                    )
                    if ci == 0:
                        # raw logits[:, 4] (global col 4 lives on core 0)
                        nc.vector.tensor_copy(l4_sb[:, j : j + 1], ps[:, COPY : COPY + 1])
                    nc.scalar.activation(
                        exp_sb[:, j, c0 : c0 + cw],
                        ps[:, :],
                        AF.Exp,
                        accum_out=part_sb[:, j, ci : ci + 1],
                    )
                    if ci == 0:
                        # core 0: exp(mod_logits)[:,COPY] = exp(1e-10) = 1.0
                        # blended as: e4 = im4*e4 + m4
                        nc.vector.scalar_tensor_tensor(
                            exp_sb[:, j, COPY : COPY + 1],
                            exp_sb[:, j, COPY : COPY + 1],
                            im4_sb[:, :],
                            m4_sb[:, :],
                            ALU.mult,
                            ALU.add,
                        )

            # ---- local stats -> AllReduce -----------------------------
            for j in range(NBT):
                nc.vector.tensor_reduce(
                    ccin_sb[:, 0, j : j + 1],
                    part_sb[:, j, :],
                    axis=mybir.AxisListType.X,
                    op=ALU.add,
                )
            nc.vector.tensor_scalar_mul(ccin_sb[:, 1, :], l4_sb[:, :], m4_sb[:, :])
            nc.vector.tensor_scalar_mul(ccin_sb[:, 2, :], exp_sb[:, :, PAD], m4_sb[:, :])

            cc_in = dram.tile([128, 3 * NBT], FP32, tag="ccin_d")
            cc_out = dram.tile([128, 3 * NBT], FP32, tag="ccout_d")
            nc.sync.dma_start(cc_in[:, :], ccin_sb[:, :, :])
            nc.gpsimd.collective_compute(
                "AllReduce",
                ALU.add,
                replica_groups=[list(range(NCORES))],
                ins=[cc_in.opt()],
                outs=[cc_out.opt()],
            )
            nc.sync.dma_start(sall_sb[:, :, :], cc_out[:, :])

            # ---- per-row coefficients [128, NBT] ----------------------
            se = sall_sb[:, 0, :]
            l4s = sall_sb[:, 1, :]
            e0s = sall_sb[:, 2, :]

            cpy = const.tile([128, NBT], FP32, tag="cpy")
            omc = const.tile([128, NBT], FP32, tag="omc")
            rse = const.tile([128, NBT], FP32, tag="rse")
            romc = const.tile([128, NBT], FP32, tag="romc")
            nrm = const.tile([128, NBT], FP32, tag="nrm")
            rno = const.tile([128, NBT], FP32, tag="rno")
            alpha = const.tile([128, NBT], FP32, tag="alpha")
            gamma = const.tile([128, NBT], FP32, tag="gamma")
            x0 = const.tile([128, NBT], FP32, tag="x0")
            t1 = const.tile([128, NBT], FP32, tag="t1")
            t2 = const.tile([128, NBT], FP32, tag="t2")

            # copy = 1/(1+exp(-l4))
            nc.scalar.activation(t1[:, :], l4s, AF.Exp, scale=-1.0)
            nc.vector.tensor_scalar_add(t2[:, :], t1[:, :], 1.0)
            nc.vector.reciprocal(cpy[:, :], t2[:, :])
            # omc = 1 - copy
            nc.vector.tensor_scalar(
                omc[:, :], cpy[:, :], -1.0, 1.0, ALU.mult, ALU.add
            )
            nc.vector.reciprocal(rse[:, :], se)
            nc.vector.reciprocal(romc[:, :], omc[:, :])
            # norm = omc*(1 - e0/se) + copy*anz + EPS
            nc.vector.tensor_mul(t1[:, :], e0s, rse[:, :])
            nc.vector.tensor_scalar(
                t1[:, :], t1[:, :], -1.0, 1.0, ALU.mult, ALU.add
            )  # 1 - prob0
            nc.vector.tensor_mul(t1[:, :], t1[:, :], omc[:, :])
            nc.vector.tensor_mul(t2[:, :], cpy[:, :], anz_sb[:, :])
            nc.vector.tensor_add(nrm[:, :], t1[:, :], t2[:, :])
            nc.vector.tensor_scalar_add(nrm[:, :], nrm[:, :], EPS)
            nc.vector.reciprocal(rno[:, :], nrm[:, :])
            # alpha = omc * rse * rno
            nc.vector.tensor_mul(alpha[:, :], omc[:, :], rse[:, :])
            nc.vector.tensor_mul(alpha[:, :], alpha[:, :], rno[:, :])
            # gamma = copy * se * romc
            nc.vector.tensor_mul(gamma[:, :], cpy[:, :], se)
            nc.vector.tensor_mul(gamma[:, :], gamma[:, :], romc[:, :])
            # x0 = EPS * se * romc   (PAD column substitute in exp domain)
            nc.vector.tensor_mul(x0[:, :], se, romc[:, :])
            nc.vector.tensor_scalar_mul(x0[:, :], x0[:, :], EPS)

            # core 0: exp_sb[:, :, PAD] = x0 ; others unchanged
            nc.vector.tensor_scalar_mul(t1[:, :], exp_sb[:, :, PAD], im4_sb[:, :])
            nc.vector.tensor_scalar_mul(t2[:, :], x0[:, :], m4_sb[:, :])
            nc.vector.tensor_add(t1[:, :], t1[:, :], t2[:, :])
            nc.vector.tensor_copy(exp_sb[:, :, PAD], t1[:, :])

            # ---- pass 2: out = ln(alpha*(exp + gamma*val) + EPS) ------
            for j in range(NBT):
                for h0, hw in HALVES:
                    vt = valp.tile([128, hw], BF16, tag="val")
                    nc.sync.dma_start(
                        vt[:, :], val_d.ap()[j * 128 : (j + 1) * 128, h0 : h0 + hw]
                    )
                    # u = val*gamma + exp   (in place into exp_sb)
                    nc.vector.scalar_tensor_tensor(
                        exp_sb[:, j, h0 : h0 + hw],
                        vt[:, :],
                        gamma[:, j : j + 1],
                        exp_sb[:, j, h0 : h0 + hw],
                        ALU.mult,
                        ALU.add,
                    )
                    st = stg.tile([128, hw], FP32, tag="stg")
                    nc.scalar.activation(
                        st[:, :],
                        exp_sb[:, j, h0 : h0 + hw],
                        AF.Ln,
                        bias=eps_sb[:, :],
                        scale=alpha[:, j : j + 1],
                    )
                    nc.sync.dma_start(
                        out_d.ap()[j * 128 : (j + 1) * 128, h0 : h0 + hw], st[:, :]
                    )

    nc.compile()
    return nc


def prep_inputs(hidden, src, attn, W, b, alignment):
    """Host-side sharding/layout prep. Returns per-core in_maps."""
    bf16 = ml_dtypes.bfloat16
    hidden = np.asarray(hidden, dtype=np.float32)
    attn = np.asarray(attn, dtype=np.float32)
    W = np.asarray(W, dtype=np.float32)
    b = np.asarray(b, dtype=np.float32)
    src = np.asarray(src).astype(np.int64)
    alignment = np.asarray(alignment).astype(np.int64)

    ht = np.ascontiguousarray(hidden.astype(bf16).T)          # [H, B]
    Wbf = W.astype(bf16)

    tgt = alignment[src]                                       # [B, S]
    val_dense = np.zeros((B, V), np.float32)
    np.add.at(val_dense, (np.arange(B)[:, None], tgt), attn)
    val_dense[:, PAD] = 0.0
    val_bf = val_dense.astype(bf16)

    anz = (attn * (tgt != PAD)).sum(axis=1).astype(np.float32)  # [B]
    anz_t = np.ascontiguousarray(anz.reshape(NBT, 128).T)       # [128, NBT]

    ones = np.ones((1, 128), dtype=bf16)

    in_maps = []
    for c in range(NCORES):
        vlo, vhi = c * VC, (c + 1) * VC
        m4 = np.full((128, 1), 1.0 if c == 0 else 0.0, np.float32)
        im4 = np.full((128, 1), 0.0 if c == 0 else 1.0, np.float32)
        in_maps.append(
            {
                "wt": np.ascontiguousarray(Wbf[vlo:vhi, :].T),
                "ht": ht,
                "bias": np.ascontiguousarray(b[vlo:vhi].astype(bf16).reshape(1, VC)),
                "val": np.ascontiguousarray(val_bf[:, vlo:vhi]),
                "anz": anz_t,
                "m4": m4,
                "im4": im4,
                "ones": ones,
            }
        )
    return in_maps


_NC_CACHE = {}


def _get_nc(debug=False):
    key = bool(debug)
    if key not in _NC_CACHE:
        _NC_CACHE[key] = build_nc(debug=debug)
    return _NC_CACHE[key]


def run(inputs, trace=False):
    """Run on hardware; returns (full_output, BassKernelResults)."""
    nc = _get_nc()
    in_maps = prep_inputs(**inputs)
    res = bass_utils.run_bass_kernel_spmd(
        nc, in_maps, core_ids=list(range(NCORES)), trace=trace
    )
    out = np.concatenate([res.results[c]["out"] for c in range(NCORES)], axis=1)
    return out, res


def kernel(**inputs) -> np.ndarray:
    out, _ = run(inputs, trace=False)
    return out


# revision 8
# speedup vs baseline: 1.0235x; 1.0235x over previous
# CopyGenerator kernel for 8 TRN2 NeuronCores (Bass/Tile, SPMD).
#
# reference computation:
#   logits = hidden @ W.T + b                      [B=1024, V=50000]
#   mod_logits = logits with col COPY(4) = 1e-10
#   prob = softmax(mod_logits); copy = sigmoid(logits[:, 4])
#   out_prob = prob*(1-copy); out_prob[b, alignment[src[b,s]]] += attn[b,s]*copy[b]
#   out_prob[:, 0] = EPS; norm = out_prob.sum(-1)
#   out = log(out_prob/norm + EPS)
#
# Strategy: tensor-parallel over the vocab dim (each core owns VC=6250 columns
# of W / the output).  Batch rows live on SBUF partitions (8 batch tiles of
# 128 rows).  Per-row softmax statistics (sum_exp, logits[:,4],
# exp(mod_logits)[:,0]) are combined across cores with a tiny AllReduce.  The
# per-row scatter-add is reformulated in the exp domain:
#   out[b,v] = ln(alpha[b]*(exp(mod_logits[b,v]) + gamma[b]*val[b,v]) + EPS)
#   alpha = (1-copy)/(sum_exp*norm), gamma = copy*sum_exp/(1-copy)
# where val[b,v] = sum_s attn[b,s]*[alignment[src[b,s]] == v] is input-only and
# precomputed (dense, bf16) on the host as part of sharding.
#
# The batch is processed in two groups of 4 batch tiles.  Group 0's
# stats/AllReduce/output pass are emitted interleaved with group 1's matmul
# pass so the TensorEngine never waits on the collective; only group 1's tail
# is exposed.  W chunks are re-streamed per group (2x W traffic, hidden under
# the matmuls).
import numpy as np
import ml_dtypes

import concourse.bacc as bacc
import concourse.bass as bass
import concourse.mybir as mybir
import concourse.tile as tile
from concourse import bass_utils

FP32 = mybir.dt.float32
BF16 = mybir.dt.bfloat16
AF = mybir.ActivationFunctionType
ALU = mybir.AluOpType

B, S, H, V = 1024, 128, 1024, 50000
NCORES = 8
VC = V // NCORES          # 6250 vocab columns per core
NBT = B // 128            # 8 batch tiles of 128 rows
KC = H // 128             # 8 contraction chunks of 128
COPY, PAD, EPS = 4, 0, 1e-10

CHUNK = 512
CHUNKS = [(i * CHUNK, CHUNK) for i in range(VC // CHUNK)]
if VC % CHUNK:
    CHUNKS.append(((VC // CHUNK) * CHUNK, VC % CHUNK))
NCH = len(CHUNKS)

# pass-2 segments; even sizes keep bf16 slices 4-byte aligned
SEGS = [(0, 1564), (1564, 1564), (3128, 1564), (4692, VC - 4692)]

GROUPS = [(0, 1, 2, 3), (4, 5, 6, 7)]


def build_nc(debug: bool = False):
    nc = bacc.Bacc(
        "TRN2", target_bir_lowering=False, debug=debug, num_devices=NCORES
    )
    wt_d = nc.dram_tensor("wt", [H, VC], BF16, kind="ExternalInput")
    ht_d = nc.dram_tensor("ht", [H, B], BF16, kind="ExternalInput")
    b_d = nc.dram_tensor("bias", [1, VC], BF16, kind="ExternalInput")
    val_d = nc.dram_tensor("val", [B, VC], BF16, kind="ExternalInput")
    anz_d = nc.dram_tensor("anz", [128, NBT], FP32, kind="ExternalInput")
    m4_d = nc.dram_tensor("m4", [128, 1], FP32, kind="ExternalInput")
    im4_d = nc.dram_tensor("im4", [128, 1], FP32, kind="ExternalInput")
    ones_d = nc.dram_tensor("ones", [1, 128], BF16, kind="ExternalInput")
    out_d = nc.dram_tensor("out", [B, VC], FP32, kind="ExternalOutput")

    wt_ap = wt_d.ap().rearrange("(k p) v -> p k v", p=128)
    ht_ap = ht_d.ap().rearrange("(k p) b -> p k b", p=128)

    with tile.TileContext(nc) as tc:
        with (
            tc.tile_pool(name="const", bufs=1) as const,
            tc.tile_pool(name="wtp", bufs=2) as wtp,
            tc.tile_pool(name="valp", bufs=3) as valp,
            tc.tile_pool(name="up", bufs=3) as up,
            tc.tile_pool(name="stg", bufs=3) as stg,
            tc.tile_pool(name="ps", bufs=4, space="PSUM") as psp,
            tc.tile_pool(name="dram", bufs=1, space="DRAM") as dram,
        ):
            # ---- resident tensors -------------------------------------
            ht_sb = const.tile([128, KC, B], BF16, tag="ht")
            nc.sync.dma_start(ht_sb[:, :, :], ht_ap)
            b_sb = const.tile([1, VC], BF16, tag="bias")
            nc.sync.dma_start(b_sb[:, :], b_d.ap())
            ones_sb = const.tile([1, 128], BF16, tag="ones")
            nc.sync.dma_start(ones_sb[:, :], ones_d.ap())
            m4_sb = const.tile([128, 1], FP32, tag="m4")
            nc.sync.dma_start(m4_sb[:, :], m4_d.ap())
            im4_sb = const.tile([128, 1], FP32, tag="im4")
            nc.sync.dma_start(im4_sb[:, :], im4_d.ap())
            anz_sb = const.tile([128, NBT], FP32, tag="anz")
            nc.sync.dma_start(anz_sb[:, :], anz_d.ap())
            eps_sb = const.tile([128, 1], FP32, tag="eps")
            nc.vector.memset(eps_sb[:, :], EPS)

            NG = len(GROUPS)
            state = []  # per-group tiles
            for g, btiles in enumerate(GROUPS):
                gb = len(btiles)
                st = dict(
                    btiles=btiles,
                    exp=const.tile([128, gb, VC], BF16, tag=f"exp{g}", name=f"exp{g}"),
                    part=const.tile([128, gb, NCH], FP32, tag=f"part{g}", name=f"part{g}"),
                    l4=const.tile([128, gb], FP32, tag=f"l4_{g}", name=f"l4_{g}"),
                    ccin=const.tile([128, 3, gb], FP32, tag=f"ccin{g}", name=f"ccin{g}"),
                    sall=const.tile([128, 3, gb], FP32, tag=f"sall{g}", name=f"sall{g}"),
                    alpha=const.tile([128, gb], FP32, tag=f"alpha{g}", name=f"alpha{g}"),
                    gamma=const.tile([128, gb], FP32, tag=f"gamma{g}", name=f"gamma{g}"),
                    t1=const.tile([128, gb], FP32, tag=f"t1_{g}", name=f"t1_{g}"),
                    t2=const.tile([128, gb], FP32, tag=f"t2_{g}", name=f"t2_{g}"),
                    t3=const.tile([128, gb], FP32, tag=f"t3_{g}", name=f"t3_{g}"),
                    cc_in=dram.tile([128, 3 * gb], FP32, tag=f"ccin_d{g}", name=f"ccin_d{g}"),
                    cc_out=dram.tile([128, 3 * gb], FP32, tag=f"ccout_d{g}", name=f"ccout_d{g}"),
                )
                state.append(st)

            def pass1_chunk(g, ci):
                st = state[g]
                c0, cw = CHUNKS[ci]
                wt_t = wtp.tile([128, KC, cw], BF16, tag="wt")
                nc.sync.dma_start(wt_t[:, :, :], wt_ap[:, :, c0 : c0 + cw])
                for jj, j in enumerate(st["btiles"]):
                    ps = psp.tile([128, cw], FP32, tag="ps")
                    for k in range(KC):
                        nc.tensor.matmul(
                            ps[:, :],
                            lhsT=ht_sb[:, k, j * 128 : (j + 1) * 128],
                            rhs=wt_t[:, k, :],
                            start=(k == 0),
                            stop=False,
                        )
                    nc.tensor.matmul(
                        ps[:, :],
                        lhsT=ones_sb[:, :],
                        rhs=b_sb[:, c0 : c0 + cw],
                        start=False,
                        stop=True,
                    )
                    if ci == 0:
                        # raw logits[:, 4] (global col 4 lives on core 0)
                        nc.vector.tensor_copy(
                            st["l4"][:, jj : jj + 1], ps[:, COPY : COPY + 1]
                        )
                    nc.scalar.activation(
                        st["exp"][:, jj, c0 : c0 + cw],
                        ps[:, :],
                        AF.Exp,
                        accum_out=st["part"][:, jj, ci : ci + 1],
                    )
                    if ci == 0:
                        # core 0: exp(mod_logits)[:,COPY] = exp(1e-10) = 1.0
                        # blended as e4 = im4*e4 + m4
                        nc.vector.scalar_tensor_tensor(
                            st["exp"][:, jj, COPY : COPY + 1],
                            st["exp"][:, jj, COPY : COPY + 1],
                            im4_sb[:, :],
                            m4_sb[:, :],
                            ALU.mult,
                            ALU.add,
                        )

            def stats_clean(g):
                st = state[g]
                gb = len(st["btiles"])
                ccin, sall = st["ccin"], st["sall"]
                for jj in range(gb):
                    nc.vector.tensor_reduce(
                        ccin[:, 0, jj : jj + 1],
                        st["part"][:, jj, :],
                        axis=mybir.AxisListType.X,
                        op=ALU.add,
                    )
                nc.vector.tensor_scalar_mul(ccin[:, 1, :], st["l4"][:, :], m4_sb[:, :])
                nc.vector.tensor_scalar_mul(
                    ccin[:, 2, :], st["exp"][:, :, PAD], m4_sb[:, :]
                )

                nc.sync.dma_start(st["cc_in"][:, :], ccin[:, :, :])
                nc.gpsimd.collective_compute(
                    "AllReduce",
                    ALU.add,
                    replica_groups=[list(range(NCORES))],
                    ins=[st["cc_in"].opt()],
                    outs=[st["cc_out"].opt()],
                )
                nc.sync.dma_start(sall[:, :, :], st["cc_out"][:, :])

                se, l4s, e0s = sall[:, 0, :], sall[:, 1, :], sall[:, 2, :]
                cpy, omc, t1 = st["t1"], st["t2"], st["t3"]
                alpha, gamma = st["alpha"], st["gamma"]
                anz_g = anz_sb[:, st["btiles"][0] : st["btiles"][0] + gb]

                nc.scalar.activation(t1[:, :], l4s, AF.Exp, scale=-1.0)
                nc.vector.tensor_scalar_add(t1[:, :], t1[:, :], 1.0)
                nc.vector.reciprocal(cpy[:, :], t1[:, :])
                nc.vector.tensor_scalar(
                    omc[:, :], cpy[:, :], -1.0, 1.0, ALU.mult, ALU.add
                )
                # gamma = cpy*se/omc
                nc.vector.reciprocal(t1[:, :], omc[:, :])  # 1/omc
                nc.vector.tensor_mul(gamma[:, :], cpy[:, :], se)
                nc.vector.tensor_mul(gamma[:, :], gamma[:, :], t1[:, :])
                # x0 = EPS*se/omc -> blend into exp[:, :, PAD] (core 0 only)
                nc.vector.tensor_mul(t1[:, :], se, t1[:, :])  # se/omc
                nc.vector.tensor_scalar_mul(t1[:, :], t1[:, :], EPS)  # x0
                nc.vector.tensor_scalar_mul(t1[:, :], t1[:, :], m4_sb[:, :])  # m4*x0
                nc.vector.tensor_scalar(
                    st["exp"][:, :, PAD],
                    st["exp"][:, :, PAD],
                    im4_sb[:, :],
                    None,
                    ALU.mult,
                )  # im4*e0 (bf16, strided)
                nc.vector.tensor_add(
                    st["exp"][:, :, PAD], st["exp"][:, :, PAD], t1[:, :]
                )
                # norm = omc*(1-e0/se) + cpy*anz + EPS
                nc.vector.reciprocal(t1[:, :], se)  # 1/se
                nc.vector.tensor_mul(t1[:, :], e0s, t1[:, :])  # e0/se
                nc.vector.tensor_scalar(
                    t1[:, :], t1[:, :], -1.0, 1.0, ALU.mult, ALU.add
                )  # 1-e0/se
                nc.vector.tensor_mul(t1[:, :], t1[:, :], omc[:, :])
                nc.vector.tensor_mul(omc[:, :], cpy[:, :], anz_g)  # cpy*anz (omc dead soon)
                nc.vector.tensor_add(t1[:, :], t1[:, :], omc[:, :])
                nc.vector.tensor_scalar_add(t1[:, :], t1[:, :], EPS)  # norm
                nc.vector.reciprocal(t1[:, :], t1[:, :])  # 1/norm
                # alpha = (1-cpy)/(se*norm) = (1-cpy) * (1/se) * (1/norm)
                nc.vector.tensor_scalar(
                    cpy[:, :], cpy[:, :], -1.0, 1.0, ALU.mult, ALU.add
                )  # omc again (cpy dead)
                nc.vector.reciprocal(alpha[:, :], se)
                nc.vector.tensor_mul(alpha[:, :], alpha[:, :], t1[:, :])
                nc.vector.tensor_mul(alpha[:, :], alpha[:, :], cpy[:, :])

            def pass2_iter(g, jj, seg):
                st = state[g]
                j = st["btiles"][jj]
                h0, hw = SEGS[seg]
                vt = valp.tile([128, hw], BF16, tag="val")
                nc.sync.dma_start(
                    vt[:, :], val_d.ap()[j * 128 : (j + 1) * 128, h0 : h0 + hw]
                )
                ut = up.tile([128, hw], BF16, tag="u")
                nc.vector.tensor_scalar_mul(
                    ut[:, :], vt[:, :], st["gamma"][:, jj : jj + 1]
                )
                nc.vector.tensor_add(
                    ut[:, :], ut[:, :], st["exp"][:, jj, h0 : h0 + hw]
                )
                stt = stg.tile([128, hw], FP32, tag="stg")
                nc.scalar.activation(
                    stt[:, :],
                    ut[:, :],
                    AF.Ln,
                    bias=eps_sb[:, :],
                    scale=st["alpha"][:, jj : jj + 1],
                )
                nc.sync.dma_start(
                    out_d.ap()[j * 128 : (j + 1) * 128, h0 : h0 + hw], stt[:, :]
                )

            # ---------------- emission schedule ------------------------
            # group 0 pass 1
            for ci in range(NCH):
                pass1_chunk(0, ci)
            # group 0 stats + collective
            stats_clean(0)
            # group 1 pass 1, interleaved with group 0 pass 2
            p2 = [(0, jj, s) for jj in range(len(GROUPS[0])) for s in range(len(SEGS))]
            p2i = 0
            for ci in range(NCH):
                pass1_chunk(1, ci)
                if ci >= 3:
                    for _ in range(2):
                        if p2i < len(p2):
                            g, jj, s = p2[p2i]
                            pass2_iter(g, jj, s)
                            p2i += 1
            while p2i < len(p2):
                g, jj, s = p2[p2i]
                pass2_iter(g, jj, s)
                p2i += 1
            # group 1 stats + pass 2
            stats_clean(1)
            for jj in range(len(GROUPS[1])):
                for s in range(len(SEGS)):
                    pass2_iter(1, jj, s)

    nc.compile()
    return nc


def prep_inputs(hidden, src, attn, W, b, alignment):
    """Host-side sharding/layout prep. Returns per-core in_maps."""
    bf16 = ml_dtypes.bfloat16
    hidden = np.asarray(hidden, dtype=np.float32)
    attn = np.asarray(attn, dtype=np.float32)
    W = np.asarray(W, dtype=np.float32)
    b = np.asarray(b, dtype=np.float32)
    src = np.asarray(src).astype(np.int64)
    alignment = np.asarray(alignment).astype(np.int64)

    ht = np.ascontiguousarray(hidden.astype(bf16).T)          # [H, B]
    Wbf = W.astype(bf16)

    tgt = alignment[src]                                       # [B, S]
    val_dense = np.zeros((B, V), np.float32)
    np.add.at(val_dense, (np.arange(B)[:, None], tgt), attn)
    val_dense[:, PAD] = 0.0
    val_bf = val_dense.astype(bf16)

    anz = (attn * (tgt != PAD)).sum(axis=1).astype(np.float32)  # [B]
    anz_t = np.ascontiguousarray(anz.reshape(NBT, 128).T)       # [128, NBT]

    ones = np.ones((1, 128), dtype=bf16)

    in_maps = []
    for c in range(NCORES):
        vlo, vhi = c * VC, (c + 1) * VC
        m4 = np.full((128, 1), 1.0 if c == 0 else 0.0, np.float32)
        im4 = np.full((128, 1), 0.0 if c == 0 else 1.0, np.float32)
        in_maps.append(
            {
                "wt": np.ascontiguousarray(Wbf[vlo:vhi, :].T),
                "ht": ht,
                "bias": np.ascontiguousarray(b[vlo:vhi].astype(bf16).reshape(1, VC)),
                "val": np.ascontiguousarray(val_bf[:, vlo:vhi]),
                "anz": anz_t,
                "m4": m4,
                "im4": im4,
                "ones": ones,
            }
        )
    return in_maps


_NC_CACHE = {}


def _get_nc(debug=False):
    key = bool(debug)
    if key not in _NC_CACHE:
        _NC_CACHE[key] = build_nc(debug=debug)
    return _NC_CACHE[key]


def run(inputs, trace=False):
    """Run on hardware; returns (full_output, BassKernelResults)."""
    nc = _get_nc()
    in_maps = prep_inputs(**inputs)
    res = bass_utils.run_bass_kernel_spmd(
        nc, in_maps, core_ids=list(range(NCORES)), trace=trace
    )
    out = np.concatenate([res.results[c]["out"] for c in range(NCORES)], axis=1)
    return out, res


def kernel(**inputs) -> np.ndarray:
    out, _ = run(inputs, trace=False)
    return out


# revision 9
# speedup vs baseline: 1.0241x; 1.0006x over previous
# CopyGenerator kernel for 8 TRN2 NeuronCores (Bass/Tile, SPMD).
#
# reference computation:
#   logits = hidden @ W.T + b                      [B=1024, V=50000]
#   mod_logits = logits with col COPY(4) = 1e-10
#   prob = softmax(mod_logits); copy = sigmoid(logits[:, 4])
#   out_prob = prob*(1-copy); out_prob[b, alignment[src[b,s]]] += attn[b,s]*copy[b]
#   out_prob[:, 0] = EPS; norm = out_prob.sum(-1)
#   out = log(out_prob/norm + EPS)
#
# Strategy: tensor-parallel over the vocab dim (each core owns VC=6250 columns
# of W / the output).  Batch rows live on SBUF partitions (8 batch tiles of
# 128 rows).  Per-row softmax statistics (sum_exp, logits[:,4],
# exp(mod_logits)[:,0]) are combined across cores with a tiny AllReduce.  The
# per-row scatter-add is reformulated in the exp domain:
#   out[b,v] = ln(alpha[b]*(exp(mod_logits[b,v]) + gamma[b]*val[b,v]) + EPS)
#   alpha = (1-copy)/(sum_exp*norm), gamma = copy*sum_exp/(1-copy)
# where val[b,v] = sum_s attn[b,s]*[alignment[src[b,s]] == v] is input-only and
# precomputed (dense, bf16) on the host as part of sharding.
#
# The batch is processed in two groups of 4 batch tiles.  Group 0's
# stats/AllReduce/output pass are emitted interleaved with group 1's matmul
# pass so the TensorEngine never waits on the collective; only group 1's tail
# is exposed.  W chunks are re-streamed per group (2x W traffic, hidden under
# the matmuls).
import numpy as np
import ml_dtypes

import concourse.bacc as bacc
import concourse.bass as bass
import concourse.mybir as mybir
import concourse.tile as tile
from concourse import bass_utils

FP32 = mybir.dt.float32
BF16 = mybir.dt.bfloat16
AF = mybir.ActivationFunctionType
ALU = mybir.AluOpType

B, S, H, V = 1024, 128, 1024, 50000
NCORES = 8
VC = V // NCORES          # 6250 vocab columns per core
NBT = B // 128            # 8 batch tiles of 128 rows
KC = H // 128             # 8 contraction chunks of 128
COPY, PAD, EPS = 4, 0, 1e-10

CHUNK = 512
CHUNKS = [(i * CHUNK, CHUNK) for i in range(VC // CHUNK)]
if VC % CHUNK:
    CHUNKS.append(((VC // CHUNK) * CHUNK, VC % CHUNK))
NCH = len(CHUNKS)

# pass-2 segments; even sizes keep bf16 slices 4-byte aligned
SEGS = [(0, 1564), (1564, 1564), (3128, 1564), (4692, VC - 4692)]

GROUPS = [(0, 1, 2, 3), (4, 5, 6, 7)]


def build_nc(debug: bool = False):
    nc = bacc.Bacc(
        "TRN2", target_bir_lowering=False, debug=debug, num_devices=NCORES
    )
    wt_d = nc.dram_tensor("wt", [H, VC], BF16, kind="ExternalInput")
    ht_d = nc.dram_tensor("ht", [H, B], BF16, kind="ExternalInput")
    b_d = nc.dram_tensor("bias", [1, VC], BF16, kind="ExternalInput")
    val_d = nc.dram_tensor("val", [B, VC], BF16, kind="ExternalInput")
    anz_d = nc.dram_tensor("anz", [128, NBT], FP32, kind="ExternalInput")
    m4_d = nc.dram_tensor("m4", [128, 1], FP32, kind="ExternalInput")
    im4_d = nc.dram_tensor("im4", [128, 1], FP32, kind="ExternalInput")
    ones_d = nc.dram_tensor("ones", [1, 128], BF16, kind="ExternalInput")
    out_d = nc.dram_tensor("out", [B, VC], FP32, kind="ExternalOutput")

    wt_ap = wt_d.ap().rearrange("(k p) v -> p k v", p=128)
    ht_ap = ht_d.ap().rearrange("(k p) b -> p k b", p=128)

    with tile.TileContext(nc) as tc:
        with (
            tc.tile_pool(name="const", bufs=1) as const,
            tc.tile_pool(name="wtp", bufs=2) as wtp,
            tc.tile_pool(name="valp", bufs=3) as valp,
            tc.tile_pool(name="up", bufs=3) as up,
            tc.tile_pool(name="stg", bufs=3) as stg,
            tc.tile_pool(name="ps", bufs=6, space="PSUM") as psp,
            tc.tile_pool(name="dram", bufs=1, space="DRAM") as dram,
        ):
            # ---- resident tensors -------------------------------------
            ht_sb = const.tile([128, KC, B], BF16, tag="ht")
            nc.sync.dma_start(ht_sb[:, :, :], ht_ap)
            b_sb = const.tile([1, VC], BF16, tag="bias")
            nc.sync.dma_start(b_sb[:, :], b_d.ap())
            ones_sb = const.tile([1, 128], BF16, tag="ones")
            nc.sync.dma_start(ones_sb[:, :], ones_d.ap())
            m4_sb = const.tile([128, 1], FP32, tag="m4")
            nc.sync.dma_start(m4_sb[:, :], m4_d.ap())
            im4_sb = const.tile([128, 1], FP32, tag="im4")
            nc.sync.dma_start(im4_sb[:, :], im4_d.ap())
            anz_sb = const.tile([128, NBT], FP32, tag="anz")
            nc.sync.dma_start(anz_sb[:, :], anz_d.ap())
            eps_sb = const.tile([128, 1], FP32, tag="eps")
            nc.vector.memset(eps_sb[:, :], EPS)

            NG = len(GROUPS)
            state = []  # per-group tiles
            for g, btiles in enumerate(GROUPS):
                gb = len(btiles)
                st = dict(
                    btiles=btiles,
                    exp=const.tile([128, gb, VC], BF16, tag=f"exp{g}", name=f"exp{g}"),
                    part=const.tile([128, gb, NCH], FP32, tag=f"part{g}", name=f"part{g}"),
                    l4=const.tile([128, gb], FP32, tag=f"l4_{g}", name=f"l4_{g}"),
                    ccin=const.tile([128, 3, gb], FP32, tag=f"ccin{g}", name=f"ccin{g}"),
                    sall=const.tile([128, 3, gb], FP32, tag=f"sall{g}", name=f"sall{g}"),
                    alpha=const.tile([128, gb], FP32, tag=f"alpha{g}", name=f"alpha{g}"),
                    gamma=const.tile([128, gb], FP32, tag=f"gamma{g}", name=f"gamma{g}"),
                    t1=const.tile([128, gb], FP32, tag=f"t1_{g}", name=f"t1_{g}"),
                    t2=const.tile([128, gb], FP32, tag=f"t2_{g}", name=f"t2_{g}"),
                    t3=const.tile([128, gb], FP32, tag=f"t3_{g}", name=f"t3_{g}"),
                    cc_in=dram.tile([128, 3 * gb], FP32, tag=f"ccin_d{g}", name=f"ccin_d{g}"),
                    cc_out=dram.tile([128, 3 * gb], FP32, tag=f"ccout_d{g}", name=f"ccout_d{g}"),
                )
                state.append(st)

            def pass1_chunk(g, ci):
                st = state[g]
                c0, cw = CHUNKS[ci]
                wt_t = wtp.tile([128, KC, cw], BF16, tag="wt")
                nc.sync.dma_start(wt_t[:, :, :], wt_ap[:, :, c0 : c0 + cw])
                for jj, j in enumerate(st["btiles"]):
                    ps = psp.tile([128, cw], FP32, tag="ps")
                    for k in range(KC):
                        nc.tensor.matmul(
                            ps[:, :],
                            lhsT=ht_sb[:, k, j * 128 : (j + 1) * 128],
                            rhs=wt_t[:, k, :],
                            start=(k == 0),
                            stop=False,
                        )
                    nc.tensor.matmul(
                        ps[:, :],
                        lhsT=ones_sb[:, :],
                        rhs=b_sb[:, c0 : c0 + cw],
                        start=False,
                        stop=True,
                    )
                    if ci == 0:
                        # raw logits[:, 4] (global col 4 lives on core 0)
                        nc.vector.tensor_copy(
                            st["l4"][:, jj : jj + 1], ps[:, COPY : COPY + 1]
                        )
                    nc.scalar.activation(
                        st["exp"][:, jj, c0 : c0 + cw],
                        ps[:, :],
                        AF.Exp,
                        accum_out=st["part"][:, jj, ci : ci + 1],
                    )
                    if ci == 0:
                        # core 0: exp(mod_logits)[:,COPY] = exp(1e-10) = 1.0
                        # blended as e4 = im4*e4 + m4
                        nc.vector.scalar_tensor_tensor(
                            st["exp"][:, jj, COPY : COPY + 1],
                            st["exp"][:, jj, COPY : COPY + 1],
                            im4_sb[:, :],
                            m4_sb[:, :],
                            ALU.mult,
                            ALU.add,
                        )

            def stats_pre(g):
                st = state[g]
                gb = len(st["btiles"])
                ccin, sall = st["ccin"], st["sall"]
                for jj in range(gb):
                    nc.vector.tensor_reduce(
                        ccin[:, 0, jj : jj + 1],
                        st["part"][:, jj, :],
                        axis=mybir.AxisListType.X,
                        op=ALU.add,
                    )
                nc.vector.tensor_scalar_mul(ccin[:, 1, :], st["l4"][:, :], m4_sb[:, :])
                nc.vector.tensor_scalar_mul(
                    ccin[:, 2, :], st["exp"][:, :, PAD], m4_sb[:, :]
                )

                nc.gpsimd.dma_start(st["cc_in"][:, :], ccin[:, :, :])
                nc.gpsimd.collective_compute(
                    "AllReduce",
                    ALU.add,
                    replica_groups=[list(range(NCORES))],
                    ins=[st["cc_in"].opt()],
                    outs=[st["cc_out"].opt()],
                )
                nc.gpsimd.dma_start(sall[:, :, :], st["cc_out"][:, :])

            def stats_post(g):
                st = state[g]
                gb = len(st["btiles"])
                sall = st["sall"]
                se, l4s, e0s = sall[:, 0, :], sall[:, 1, :], sall[:, 2, :]
                cpy, omc, t1 = st["t1"], st["t2"], st["t3"]
                alpha, gamma = st["alpha"], st["gamma"]
                anz_g = anz_sb[:, st["btiles"][0] : st["btiles"][0] + gb]

                nc.scalar.activation(t1[:, :], l4s, AF.Exp, scale=-1.0)
                nc.vector.tensor_scalar_add(t1[:, :], t1[:, :], 1.0)
                nc.vector.reciprocal(cpy[:, :], t1[:, :])
                nc.vector.tensor_scalar(
                    omc[:, :], cpy[:, :], -1.0, 1.0, ALU.mult, ALU.add
                )
                # gamma = cpy*se/omc
                nc.vector.reciprocal(t1[:, :], omc[:, :])  # 1/omc
                nc.vector.tensor_mul(gamma[:, :], cpy[:, :], se)
                nc.vector.tensor_mul(gamma[:, :], gamma[:, :], t1[:, :])
                # x0 = EPS*se/omc -> blend into exp[:, :, PAD] (core 0 only)
                nc.vector.tensor_mul(t1[:, :], se, t1[:, :])  # se/omc
                nc.vector.tensor_scalar_mul(t1[:, :], t1[:, :], EPS)  # x0
                nc.vector.tensor_scalar_mul(t1[:, :], t1[:, :], m4_sb[:, :])  # m4*x0
                nc.vector.tensor_scalar(
                    st["exp"][:, :, PAD],
                    st["exp"][:, :, PAD],
                    im4_sb[:, :],
                    None,
                    ALU.mult,
                )  # im4*e0 (bf16, strided)
                nc.vector.tensor_add(
                    st["exp"][:, :, PAD], st["exp"][:, :, PAD], t1[:, :]
                )
                # norm = omc*(1-e0/se) + cpy*anz + EPS
                nc.vector.reciprocal(t1[:, :], se)  # 1/se
                nc.vector.tensor_mul(t1[:, :], e0s, t1[:, :])  # e0/se
                nc.vector.tensor_scalar(
                    t1[:, :], t1[:, :], -1.0, 1.0, ALU.mult, ALU.add
                )  # 1-e0/se
                nc.vector.tensor_mul(t1[:, :], t1[:, :], omc[:, :])
                nc.vector.tensor_mul(omc[:, :], cpy[:, :], anz_g)  # cpy*anz (omc dead soon)
                nc.vector.tensor_add(t1[:, :], t1[:, :], omc[:, :])
                nc.vector.tensor_scalar_add(t1[:, :], t1[:, :], EPS)  # norm
                nc.vector.reciprocal(t1[:, :], t1[:, :])  # 1/norm
                # alpha = (1-cpy)/(se*norm) = (1-cpy) * (1/se) * (1/norm)
                nc.vector.tensor_scalar(
                    cpy[:, :], cpy[:, :], -1.0, 1.0, ALU.mult, ALU.add
                )  # omc again (cpy dead)
                nc.vector.reciprocal(alpha[:, :], se)
                nc.vector.tensor_mul(alpha[:, :], alpha[:, :], t1[:, :])
                nc.vector.tensor_mul(alpha[:, :], alpha[:, :], cpy[:, :])

            def pass2_iter(g, jj, seg):
                st = state[g]
                j = st["btiles"][jj]
                h0, hw = SEGS[seg]
                vt = valp.tile([128, hw], BF16, tag="val")
                nc.sync.dma_start(
                    vt[:, :], val_d.ap()[j * 128 : (j + 1) * 128, h0 : h0 + hw]
                )
                ut = up.tile([128, hw], BF16, tag="u")
                nc.vector.tensor_scalar_mul(
                    ut[:, :], vt[:, :], st["gamma"][:, jj : jj + 1]
                )
                nc.vector.tensor_add(
                    ut[:, :], ut[:, :], st["exp"][:, jj, h0 : h0 + hw]
                )
                stt = stg.tile([128, hw], FP32, tag="stg")
                nc.scalar.activation(
                    stt[:, :],
                    ut[:, :],
                    AF.Ln,
                    bias=eps_sb[:, :],
                    scale=st["alpha"][:, jj : jj + 1],
                )
                nc.sync.dma_start(
                    out_d.ap()[j * 128 : (j + 1) * 128, h0 : h0 + hw], stt[:, :]
                )

            # ---------------- emission schedule ------------------------
            # group 0 pass 1
            for ci in range(NCH):
                pass1_chunk(0, ci)
            # group 0 reduction + collective (all on gpsimd queue: non-blocking)
            stats_pre(0)
            # group 1 pass 1; group-0 coefficient math lands after chunk 3
            # (collective done by then); pass 2 of group 0 interleaves after
            p2 = [(0, jj, s) for jj in range(len(GROUPS[0])) for s in range(len(SEGS))]
            p2i = 0
            for ci in range(NCH):
                pass1_chunk(1, ci)
                if ci == 3:
                    stats_post(0)
                if ci >= 4:
                    for _ in range(2):
                        if p2i < len(p2):
                            g, jj, s = p2[p2i]
                            pass2_iter(g, jj, s)
                            p2i += 1
            while p2i < len(p2):
                g, jj, s = p2[p2i]
                pass2_iter(g, jj, s)
                p2i += 1
            # group 1 stats + pass 2
            stats_pre(1)
            stats_post(1)
            for jj in range(len(GROUPS[1])):
                for s in range(len(SEGS)):
                    pass2_iter(1, jj, s)

    nc.compile()
    return nc


def prep_inputs(hidden, src, attn, W, b, alignment):
    """Host-side sharding/layout prep. Returns per-core in_maps."""
    bf16 = ml_dtypes.bfloat16
    hidden = np.asarray(hidden, dtype=np.float32)
    attn = np.asarray(attn, dtype=np.float32)
    W = np.asarray(W, dtype=np.float32)
    b = np.asarray(b, dtype=np.float32)
    src = np.asarray(src).astype(np.int64)
    alignment = np.asarray(alignment).astype(np.int64)

    ht = np.ascontiguousarray(hidden.astype(bf16).T)          # [H, B]
    Wbf = W.astype(bf16)

    tgt = alignment[src]                                       # [B, S]
    val_dense = np.zeros((B, V), np.float32)
    np.add.at(val_dense, (np.arange(B)[:, None], tgt), attn)
    val_dense[:, PAD] = 0.0
    val_bf = val_dense.astype(bf16)

    anz = (attn * (tgt != PAD)).sum(axis=1).astype(np.float32)  # [B]
    anz_t = np.ascontiguousarray(anz.reshape(NBT, 128).T)       # [128, NBT]

    ones = np.ones((1, 128), dtype=bf16)

    in_maps = []
    for c in range(NCORES):
        vlo, vhi = c * VC, (c + 1) * VC
        m4 = np.full((128, 1), 1.0 if c == 0 else 0.0, np.float32)
        im4 = np.full((128, 1), 0.0 if c == 0 else 1.0, np.float32)
        in_maps.append(
            {
                "wt": np.ascontiguousarray(Wbf[vlo:vhi, :].T),
                "ht": ht,
                "bias": np.ascontiguousarray(b[vlo:vhi].astype(bf16).reshape(1, VC)),
                "val": np.ascontiguousarray(val_bf[:, vlo:vhi]),
                "anz": anz_t,
                "m4": m4,
                "im4": im4,
                "ones": ones,
            }
        )
    return in_maps


_NC_CACHE = {}


def _get_nc(debug=False):
    key = bool(debug)
    if key not in _NC_CACHE:
        _NC_CACHE[key] = build_nc(debug=debug)
    return _NC_CACHE[key]


def run(inputs, trace=False):
    """Run on hardware; returns (full_output, BassKernelResults)."""
    nc = _get_nc()
    in_maps = prep_inputs(**inputs)
    res = bass_utils.run_bass_kernel_spmd(
        nc, in_maps, core_ids=list(range(NCORES)), trace=trace
    )
    out = np.concatenate([res.results[c]["out"] for c in range(NCORES)], axis=1)
    return out, res


def kernel(**inputs) -> np.ndarray:
    out, _ = run(inputs, trace=False)
    return out


# revision 10
# speedup vs baseline: 1.2724x; 1.2424x over previous
# CopyGenerator kernel for 8 TRN2 NeuronCores (Bass/Tile, SPMD).
#
# reference computation:
#   logits = hidden @ W.T + b                      [B=1024, V=50000]
#   mod_logits = logits with col COPY(4) = 1e-10
#   prob = softmax(mod_logits); copy = sigmoid(logits[:, 4])
#   out_prob = prob*(1-copy); out_prob[b, alignment[src[b,s]]] += attn[b,s]*copy[b]
#   out_prob[:, 0] = EPS; norm = out_prob.sum(-1)
#   out = log(out_prob/norm + EPS)
#
# Strategy: tensor-parallel over the vocab dim (each core owns VC=6250 columns
# of W / the output).  Batch rows live on SBUF partitions (8 batch tiles of
# 128 rows).  Per-row softmax statistics (sum_exp, logits[:,4],
# exp(mod_logits)[:,0]) are combined across cores with a tiny AllReduce.  The
# per-row scatter-add is reformulated in the exp domain:
#   out[b,v] = ln(alpha[b]*(exp(mod_logits[b,v]) + gamma[b]*val[b,v]) + EPS)
#   alpha = (1-copy)/(sum_exp*norm), gamma = copy*sum_exp/(1-copy)
# where val[b,v] = sum_s attn[b,s]*[alignment[src[b,s]] == v] is input-only and
# precomputed (dense, bf16) on the host as part of sharding.
#
# The batch is processed in groups of batch tiles.  Each group's
# stats/AllReduce/output pass is emitted interleaved with the next group's
# matmul pass so the TensorEngine never waits on a collective; only the last
# group's tail is exposed.  W chunks are re-streamed per group (hidden under
# the matmuls).  The matmul runs in fp8 (e4m3) with DoubleRow packing
# (K=256 per matmul); the bias row is added with a separate K=1 bf16 matmul
# into the same PSUM accumulation group.
import numpy as np
import ml_dtypes

import concourse.bacc as bacc
import concourse.bass as bass
import concourse.mybir as mybir
import concourse.tile as tile
from concourse import bass_utils

FP32 = mybir.dt.float32
BF16 = mybir.dt.bfloat16
FP8 = mybir.dt.float8e4
AF = mybir.ActivationFunctionType
ALU = mybir.AluOpType

B, S, H, V = 1024, 128, 1024, 50000
NCORES = 8
VC = V // NCORES          # 6250 vocab columns per core
NBT = B // 128            # 8 batch tiles of 128 rows
KC = H // 128             # 8 contraction chunks of 128
KD = KC // 2              # 4 DoubleRow chunks of 256
COPY, PAD, EPS = 4, 0, 1e-10

USE_FP8 = True

CHUNK = 512
CHUNKS = [(i * CHUNK, CHUNK) for i in range(VC // CHUNK)]
if VC % CHUNK:
    CHUNKS.append(((VC // CHUNK) * CHUNK, VC % CHUNK))
NCH = len(CHUNKS)

# pass-2 segments; even sizes keep bf16 slices 4-byte aligned
SEGS = [(0, 1564), (1564, 1564), (3128, 1564), (4692, VC - 4692)]

GROUPS = [(0, 1, 2), (3, 4, 5), (6, 7)]


def build_nc(debug: bool = False):
    nc = bacc.Bacc(
        "TRN2", target_bir_lowering=False, debug=debug, num_devices=NCORES
    )
    wdt = FP8 if USE_FP8 else BF16
    wt_d = nc.dram_tensor("wt", [H, VC], wdt, kind="ExternalInput")
    ht_d = nc.dram_tensor("ht", [H, B], wdt, kind="ExternalInput")
    b_d = nc.dram_tensor("bias", [1, VC], BF16, kind="ExternalInput")
    val_d = nc.dram_tensor("val", [B, VC], BF16, kind="ExternalInput")
    anz_d = nc.dram_tensor("anz", [128, NBT], FP32, kind="ExternalInput")
    m4_d = nc.dram_tensor("m4", [128, 1], FP32, kind="ExternalInput")
    im4_d = nc.dram_tensor("im4", [128, 1], FP32, kind="ExternalInput")
    ones_d = nc.dram_tensor("ones", [1, 128], BF16, kind="ExternalInput")
    out_d = nc.dram_tensor("out", [B, VC], FP32, kind="ExternalOutput")

    if USE_FP8:
        # DoubleRow layout: [p, kk, t, x] with contraction row = (2*kk+t)*128+p
        wt_ap = wt_d.ap().rearrange("(a t p) v -> p a t v", a=KD, t=2)
        ht_ap = ht_d.ap().rearrange("(a t p) b -> p a t b", a=KD, t=2)
    else:
        wt_ap = wt_d.ap().rearrange("(k p) v -> p k v", p=128)
        ht_ap = ht_d.ap().rearrange("(k p) b -> p k b", p=128)

    with tile.TileContext(nc) as tc:
        with (
            tc.tile_pool(name="const", bufs=1) as const,
            tc.tile_pool(name="wtp", bufs=2) as wtp,
            tc.tile_pool(name="valp", bufs=8) as valp,
            tc.tile_pool(name="up", bufs=4) as up,
            tc.tile_pool(name="stg", bufs=4) as stg,
            tc.tile_pool(name="ps", bufs=6, space="PSUM") as psp,
            tc.tile_pool(name="dram", bufs=1, space="DRAM") as dram,
        ):
            # ---- resident tensors -------------------------------------
            if USE_FP8:
                ht_sb = const.tile([128, KD, 2, B], FP8, tag="ht", name="ht_sb")
            else:
                ht_sb = const.tile([128, KC, B], BF16, tag="ht", name="ht_sb")
            nc.sync.dma_start(ht_sb[:, :, :], ht_ap)
            b_sb = const.tile([1, VC], BF16, tag="bias", name="b_sb")
            nc.sync.dma_start(b_sb[:, :], b_d.ap())
            ones_sb = const.tile([1, 128], BF16, tag="ones", name="ones_sb")
            nc.sync.dma_start(ones_sb[:, :], ones_d.ap())
            m4_sb = const.tile([128, 1], FP32, tag="m4", name="m4_sb")
            nc.sync.dma_start(m4_sb[:, :], m4_d.ap())
            im4_sb = const.tile([128, 1], FP32, tag="im4", name="im4_sb")
            nc.sync.dma_start(im4_sb[:, :], im4_d.ap())
            anz_sb = const.tile([128, NBT], FP32, tag="anz", name="anz_sb")
            nc.sync.dma_start(anz_sb[:, :], anz_d.ap())
            eps_sb = const.tile([128, 1], FP32, tag="eps", name="eps_sb")
            nc.vector.memset(eps_sb[:, :], EPS)

            state = []  # per-group tiles
            for g, btiles in enumerate(GROUPS):
                gb = len(btiles)
                st = dict(
                    btiles=btiles,
                    exp=const.tile([128, gb, VC], BF16, tag=f"exp{g}", name=f"exp{g}"),
                    part=const.tile(
                        [128, gb, NCH], FP32, tag=f"part{g}", name=f"part{g}"
                    ),
                    l4=const.tile([128, gb], FP32, tag=f"l4_{g}", name=f"l4_{g}"),
                    ccin=const.tile(
                        [128, 3, gb], FP32, tag=f"ccin{g}", name=f"ccin{g}"
                    ),
                    sall=const.tile(
                        [128, 3, gb], FP32, tag=f"sall{g}", name=f"sall{g}"
                    ),
                    alpha=const.tile(
                        [128, gb], FP32, tag=f"alpha{g}", name=f"alpha{g}"
                    ),
                    gamma=const.tile(
                        [128, gb], FP32, tag=f"gamma{g}", name=f"gamma{g}"
                    ),
                    t1=const.tile([128, gb], FP32, tag=f"t1_{g}", name=f"t1_{g}"),
                    t2=const.tile([128, gb], FP32, tag=f"t2_{g}", name=f"t2_{g}"),
                    t3=const.tile([128, gb], FP32, tag=f"t3_{g}", name=f"t3_{g}"),
                    cc_in=dram.tile(
                        [128, 3 * gb], FP32, tag=f"ccin_d{g}", name=f"ccin_d{g}"
                    ),
                    cc_out=dram.tile(
                        [128, 3 * gb], FP32, tag=f"ccout_d{g}", name=f"ccout_d{g}"
                    ),
                )
                state.append(st)

            def pass1_chunk(g, ci):
                st = state[g]
                c0, cw = CHUNKS[ci]
                if USE_FP8:
                    wt_t = wtp.tile([128, KD, 2, cw], FP8, tag="wt", name="wt_t")
                    nc.sync.dma_start(wt_t[:, :, :, :], wt_ap[:, :, :, c0 : c0 + cw])
                else:
                    wt_t = wtp.tile([128, KC, cw], BF16, tag="wt", name="wt_t")
                    nc.sync.dma_start(wt_t[:, :, :], wt_ap[:, :, c0 : c0 + cw])
                for jj, j in enumerate(st["btiles"]):
                    ps = psp.tile([128, cw], FP32, tag="ps", name="ps")
                    if USE_FP8:
                        for kk in range(KD):
                            nc.tensor.matmul(
                                ps[:, :],
                                lhsT=ht_sb[:, kk, :, j * 128 : (j + 1) * 128],
                                rhs=wt_t[:, kk, :, :],
                                start=(kk == 0),
                                stop=False,
                                perf_mode=mybir.MatmulPerfMode.DoubleRow,
                            )
                    else:
                        for k in range(KC):
                            nc.tensor.matmul(
                                ps[:, :],
                                lhsT=ht_sb[:, k, j * 128 : (j + 1) * 128],
                                rhs=wt_t[:, k, :],
                                start=(k == 0),
                                stop=False,
                            )
                    nc.tensor.matmul(
                        ps[:, :],
                        lhsT=ones_sb[:, :],
                        rhs=b_sb[:, c0 : c0 + cw],
                        start=False,
                        stop=True,
                    )
                    if ci == 0:
                        # raw logits[:, 4] (global col 4 lives on core 0)
                        nc.vector.tensor_copy(
                            st["l4"][:, jj : jj + 1], ps[:, COPY : COPY + 1]
                        )
                    nc.scalar.activation(
                        st["exp"][:, jj, c0 : c0 + cw],
                        ps[:, :],
                        AF.Exp,
                        accum_out=st["part"][:, jj, ci : ci + 1],
                    )
                    if ci == 0:
                        # core 0: exp(mod_logits)[:,COPY] = exp(1e-10) = 1.0
                        # blended as e4 = im4*e4 + m4
                        nc.vector.scalar_tensor_tensor(
                            st["exp"][:, jj, COPY : COPY + 1],
                            st["exp"][:, jj, COPY : COPY + 1],
                            im4_sb[:, :],
                            m4_sb[:, :],
                            ALU.mult,
                            ALU.add,
                        )

            def stats_pre(g):
                """Partial-sum reduction + AllReduce; the blockable pieces sit
                on the gpsimd queue so other engines stay free."""
                st = state[g]
                gb = len(st["btiles"])
                ccin = st["ccin"]
                for jj in range(gb):
                    nc.vector.tensor_reduce(
                        ccin[:, 0, jj : jj + 1],
                        st["part"][:, jj, :],
                        axis=mybir.AxisListType.X,
                        op=ALU.add,
                    )
                nc.vector.tensor_scalar_mul(ccin[:, 1, :], st["l4"][:, :], m4_sb[:, :])
                nc.vector.tensor_scalar_mul(
                    ccin[:, 2, :], st["exp"][:, :, PAD], m4_sb[:, :]
                )
                nc.gpsimd.dma_start(st["cc_in"][:, :], ccin[:, :, :])
                nc.gpsimd.collective_compute(
                    "AllReduce",
                    ALU.add,
                    replica_groups=[list(range(NCORES))],
                    ins=[st["cc_in"].opt()],
                    outs=[st["cc_out"].opt()],
                )
                nc.gpsimd.dma_start(st["sall"][:, :, :], st["cc_out"][:, :])

            def stats_post(g):
                """Per-row coefficients from the reduced stats (DVE/ACT)."""
                st = state[g]
                gb = len(st["btiles"])
                sall = st["sall"]
                se, l4s, e0s = sall[:, 0, :], sall[:, 1, :], sall[:, 2, :]
                cpy, omc, t1 = st["t1"], st["t2"], st["t3"]
                alpha, gamma = st["alpha"], st["gamma"]
                anz_g = anz_sb[:, st["btiles"][0] : st["btiles"][0] + gb]

                nc.scalar.activation(t1[:, :], l4s, AF.Exp, scale=-1.0)
                nc.vector.tensor_scalar_add(t1[:, :], t1[:, :], 1.0)
                nc.vector.reciprocal(cpy[:, :], t1[:, :])
                nc.vector.tensor_scalar(
                    omc[:, :], cpy[:, :], -1.0, 1.0, ALU.mult, ALU.add
                )
                # gamma = cpy*se/omc
                nc.vector.reciprocal(t1[:, :], omc[:, :])  # 1/omc
                nc.vector.tensor_mul(gamma[:, :], cpy[:, :], se)
                nc.vector.tensor_mul(gamma[:, :], gamma[:, :], t1[:, :])
                # x0 = EPS*se/omc -> blend into exp[:, :, PAD] (core 0 only)
                nc.vector.tensor_mul(t1[:, :], se, t1[:, :])  # se/omc
                nc.vector.tensor_scalar_mul(t1[:, :], t1[:, :], EPS)  # x0
                nc.vector.tensor_scalar_mul(t1[:, :], t1[:, :], m4_sb[:, :])  # m4*x0
                nc.vector.tensor_scalar(
                    st["exp"][:, :, PAD],
                    st["exp"][:, :, PAD],
                    im4_sb[:, :],
                    None,
                    ALU.mult,
                )  # im4*e0 (bf16, strided)
                nc.vector.tensor_add(
                    st["exp"][:, :, PAD], st["exp"][:, :, PAD], t1[:, :]
                )
                # norm = omc*(1-e0/se) + cpy*anz + EPS
                nc.vector.reciprocal(t1[:, :], se)  # 1/se
                nc.vector.tensor_mul(t1[:, :], e0s, t1[:, :])  # e0/se
                nc.vector.tensor_scalar(
                    t1[:, :], t1[:, :], -1.0, 1.0, ALU.mult, ALU.add
                )  # 1-e0/se
                nc.vector.tensor_mul(t1[:, :], t1[:, :], omc[:, :])
                nc.vector.tensor_mul(omc[:, :], cpy[:, :], anz_g)  # cpy*anz
                nc.vector.tensor_add(t1[:, :], t1[:, :], omc[:, :])
                nc.vector.tensor_scalar_add(t1[:, :], t1[:, :], EPS)  # norm
                nc.vector.reciprocal(t1[:, :], t1[:, :])  # 1/norm
                # alpha = (1-cpy) * (1/se) * (1/norm)
                nc.vector.tensor_scalar(
                    cpy[:, :], cpy[:, :], -1.0, 1.0, ALU.mult, ALU.add
                )  # omc again
                nc.vector.reciprocal(alpha[:, :], se)
                nc.vector.tensor_mul(alpha[:, :], alpha[:, :], t1[:, :])
                nc.vector.tensor_mul(alpha[:, :], alpha[:, :], cpy[:, :])

            def pass2_iter(g, jj, seg):
                st = state[g]
                j = st["btiles"][jj]
                h0, hw = SEGS[seg]
                vt = valp.tile([128, hw], BF16, tag="val", name="vt")
                nc.sync.dma_start(
                    vt[:, :], val_d.ap()[j * 128 : (j + 1) * 128, h0 : h0 + hw]
                )
                ut = up.tile([128, hw], BF16, tag="u", name="ut")
                nc.vector.tensor_scalar_mul(
                    ut[:, :], vt[:, :], st["gamma"][:, jj : jj + 1]
                )
                nc.vector.tensor_add(
                    ut[:, :], ut[:, :], st["exp"][:, jj, h0 : h0 + hw]
                )
                stt = stg.tile([128, hw], FP32, tag="stg", name="stt")
                nc.scalar.activation(
                    stt[:, :],
                    ut[:, :],
                    AF.Ln,
                    bias=eps_sb[:, :],
                    scale=st["alpha"][:, jj : jj + 1],
                )
                nc.sync.dma_start(
                    out_d.ap()[j * 128 : (j + 1) * 128, h0 : h0 + hw], stt[:, :]
                )

            # ---------------- emission schedule ------------------------
            NG = len(GROUPS)
            pending = []  # deferred pass-2 iterations of the previous group
            for g in range(NG):
                for ci in range(NCH):
                    pass1_chunk(g, ci)
                    if g > 0:
                        if ci == 2:
                            stats_post(g - 1)
                        if ci >= 3:
                            for _ in range(2):
                                if pending:
                                    pass2_iter(*pending.pop(0))
                # leftover pass-2 of the previous group (if any)
                while pending:
                    pass2_iter(*pending.pop(0))
                stats_pre(g)
                pending = [
                    (g, jj, s)
                    for jj in range(len(GROUPS[g]))
                    for s in range(len(SEGS))
                ]
            # exposed tail: last group's coefficients + output pass
            stats_post(NG - 1)
            while pending:
                pass2_iter(*pending.pop(0))

    nc.compile()
    return nc


def prep_inputs(hidden, src, attn, W, b, alignment):
    """Host-side sharding/layout prep. Returns per-core in_maps."""
    bf16 = ml_dtypes.bfloat16
    wnp = ml_dtypes.float8_e4m3 if USE_FP8 else bf16
    hidden = np.asarray(hidden, dtype=np.float32)
    attn = np.asarray(attn, dtype=np.float32)
    W = np.asarray(W, dtype=np.float32)
    b = np.asarray(b, dtype=np.float32)
    src = np.asarray(src).astype(np.int64)
    alignment = np.asarray(alignment).astype(np.int64)

    ht = np.ascontiguousarray(hidden.astype(wnp).T)          # [H, B]
    Wq = W.astype(wnp)

    tgt = alignment[src]                                       # [B, S]
    val_dense = np.zeros((B, V), np.float32)
    np.add.at(val_dense, (np.arange(B)[:, None], tgt), attn)
    val_dense[:, PAD] = 0.0
    val_bf = val_dense.astype(bf16)

    anz = (attn * (tgt != PAD)).sum(axis=1).astype(np.float32)  # [B]
    anz_t = np.ascontiguousarray(anz.reshape(NBT, 128).T)       # [128, NBT]

    ones = np.ones((1, 128), dtype=bf16)

    in_maps = []
    for c in range(NCORES):
        vlo, vhi = c * VC, (c + 1) * VC
        m4 = np.full((128, 1), 1.0 if c == 0 else 0.0, np.float32)
        im4 = np.full((128, 1), 0.0 if c == 0 else 1.0, np.float32)
        in_maps.append(
            {
                "wt": np.ascontiguousarray(Wq[vlo:vhi, :].T),
                "ht": ht,
                "bias": np.ascontiguousarray(b[vlo:vhi].astype(bf16).reshape(1, VC)),
                "val": np.ascontiguousarray(val_bf[:, vlo:vhi]),
                "anz": anz_t,
                "m4": m4,
                "im4": im4,
                "ones": ones,
            }
        )
    return in_maps


_NC_CACHE = {}


def _get_nc(debug=False):
    key = bool(debug)
    if key not in _NC_CACHE:
        _NC_CACHE[key] = build_nc(debug=debug)
    return _NC_CACHE[key]


def run(inputs, trace=False):
    """Run on hardware; returns (full_output, BassKernelResults)."""
    nc = _get_nc()
    in_maps = prep_inputs(**inputs)
    res = bass_utils.run_bass_kernel_spmd(
        nc, in_maps, core_ids=list(range(NCORES)), trace=trace
    )
    out = np.concatenate([res.results[c]["out"] for c in range(NCORES)], axis=1)
    return out, res


def kernel(**inputs) -> np.ndarray:
    out, _ = run(inputs, trace=False)
    return out
